# revision 1
# baseline (speedup 1.0000x reference)
"""Trainium2 Bass kernel for nn_DataAugmentation_46823733461007 (8 NeuronCores).

Reference pipeline per sample: hflip, +0.625*noise, *brightness, perspective
warp (bilinear), rotation warp (bilinear), 190x190 crop, bicubic resize to
224x224.

Mapping:
  host  - folds flip/noise/brightness into a fp16 image (even + odd-shifted
          pair arrays), solves the perspective homography, and precomputes for
          both warps per-pixel pair-gather indices + premultiplied blend
          weights (exact decomposition of the reference bilinear incl.
          zero-fill validity), plus per-sample crop-fused bicubic matrices.
  device- per warp: row-band loads -> gpsimd ap_gather (d=2 pairs; 8 samples
          per instruction via the 16-partition index groups) -> DVE blend ->
          staged to DRAM; then two PE matmuls per channel apply
          crop+bicubic-resize; data-parallel over 8 cores, 32 samples each.
"""
import sys
sys.path.insert(0, '/opt/trn_rl_repo')
import numpy as np

B, C, H, W = 256, 3, 224, 224
CROP = 190
NCORES = 8
SPC = B // NCORES          # 32 samples per core
ROUNDS = SPC // 8          # 4 rounds x 8 samples
NBAND = 7
RB = H // NBAND            # 32 output rows per band
BAND_ROWS = RB + 75        # 107 source rows held per band
NCH = 4                    # chunks per band
CROWS = RB // NCH          # 8 rows per chunk
NI = CROWS * W             # 1792 indices per chunk per group
NPAIR = W // 2             # 112 pairs per row per parity
NE = 2 * BAND_ROWS * NPAIR # ap_gather num_elems = 23968


# ------------------------------------------------------------------- host
_XG, _YG = np.meshgrid(np.arange(W, dtype=np.float32) + 0.5,
                       np.arange(H, dtype=np.float32) + 0.5, indexing='xy')


def _persp_coeffs(ep_raw_s):
    offs = np.array([[0., 0.], [195., 0.], [195., 195.], [0., 195.]], np.float32)
    start = np.array([[0., 0.], [223., 0.], [223., 223.], [0., 223.]], np.float32)
    end = ep_raw_s.astype(np.float32) + offs
    ex, ey = end[:, 0], end[:, 1]
    sx, sy = start[:, 0], start[:, 1]
    o = np.ones(4, np.float32); z = np.zeros(4, np.float32)
    r1 = np.stack([ex, ey, o, z, z, z, -sx * ex, -sx * ey], axis=-1)
    r2 = np.stack([z, z, z, ex, ey, o, -sy * ex, -sy * ey], axis=-1)
    A = np.concatenate([r1, r2], axis=0).astype(np.float32)
    b = np.concatenate([sx, sy], axis=0).astype(np.float32)
    return np.linalg.solve(A, b).astype(np.float32)


def persp_grid(ep_raw_s):
    c = _persp_coeffs(ep_raw_s)
    a, b, cc, d, e, f, g, h = [np.float32(c[i]) for i in range(8)]
    den = g * _XG + h * _YG + np.float32(1.0)
    sx = (a * _XG + b * _YG + cc) / den - np.float32(0.5)
    sy = (d * _XG + e * _YG + f) / den - np.float32(0.5)
    return sx.astype(np.float32), sy.astype(np.float32)


def rot_grid(angle):
    th = (np.float32(angle) - np.float32(16.0)) * np.float32(np.pi / 180.0)
    cos = np.float32(np.cos(th)); sin = np.float32(np.sin(th))
    cx = np.float32((W - 1) / 2.0); cy = np.float32((H - 1) / 2.0)
    dx = (_XG - np.float32(0.5)) - cx
    dy = (_YG - np.float32(0.5)) - cy
    rx = (cos * dx + sin * dy + cx).astype(np.float32)
    ry = (-sin * dx + cos * dy + cy).astype(np.float32)
    return rx, ry


def warp_fields(sx, sy):
    """Exact pair-gather decomposition of the reference `_bilinear`."""
    x0 = np.floor(sx); y0 = np.floor(sy)
    wx = (sx - x0).astype(np.float32); wy = (sy - y0).astype(np.float32)
    x0i = x0.astype(np.int64); y0i = y0.astype(np.int64)

    vx0 = ((x0i >= 0) & (x0i < W)).astype(np.float32)
    vx1 = (x0i + 1 < W).astype(np.float32) * (x0i + 1 >= 0)
    vy0 = ((y0i >= 0) & (y0i < H)).astype(np.float32)
    vy1 = (y0i + 1 < H).astype(np.float32) * (y0i + 1 >= 0)

    y0c = np.clip(y0i, 0, H - 1)
    y1c = np.clip(y0i + 1, 0, H - 1)
    x0c = np.clip(x0i, 0, W - 1)
    x1c = np.clip(x0i + 1, 0, W - 1)
    px = np.clip(x0i, 0, W - 2)

    w_e0 = np.where(x0c == px, (1 - wx) * vx0, 0.0).astype(np.float32) \
         + np.where(x1c == px, wx * vx1, 0.0).astype(np.float32)
    w_e1 = np.where(x0c == px + 1, (1 - wx) * vx0, 0.0).astype(np.float32) \
         + np.where(x1c == px + 1, wx * vx1, 0.0).astype(np.float32)

    wy0 = ((1 - wy) * vy0).astype(np.float32)
    wy1 = (wy * vy1).astype(np.float32)
    return y0c, y1c, px, w_e0, w_e1, wy0, wy1


def band_r0(b):
    return min(max(RB * b - 37, 0), H - BAND_ROWS)


def pack_warp(sx, sy):
    """-> idxA, idxB: [NBAND*NCH, NI] int16 (band-window-relative pair idx);
       wtA, wtB: [NBAND*NCH, NI, 2] fp16 premultiplied blend weights."""
    y0c, y1c, px, w_e0, w_e1, wy0, wy1 = warp_fields(sx, sy)
    eo = (px & 1).astype(np.int64)
    pr = (px >> 1).astype(np.int64)
    iA = np.empty((H, W), np.int64)
    iB = np.empty((H, W), np.int64)
    for b in range(NBAND):
        r0 = band_r0(b)
        rs = slice(RB * b, RB * (b + 1))
        iA[rs] = eo[rs] * (BAND_ROWS * NPAIR) + (y0c[rs] - r0) * NPAIR + pr[rs]
        iB[rs] = eo[rs] * (BAND_ROWS * NPAIR) + (y1c[rs] - r0) * NPAIR + pr[rs]
    assert iA.min() >= 0 and iA.max() < NE and iB.min() >= 0 and iB.max() < NE
    idxA = iA.reshape(NBAND * NCH, NI).astype(np.int16)
    idxB = iB.reshape(NBAND * NCH, NI).astype(np.int16)
    wtA = np.stack([wy0 * w_e0, wy0 * w_e1], axis=-1).reshape(NBAND * NCH, NI, 2).astype(np.float16)
    wtB = np.stack([wy1 * w_e0, wy1 * w_e1], axis=-1).reshape(NBAND * NCH, NI, 2).astype(np.float16)
    return idxA, idxB, wtA, wtB


def wrap16(u):
    """[NI] -> [16, NI//16] wrapped layout for one group."""
    return u.reshape(NI // 16, 16).T


def bicubic_weight_mat(n_in, n_out):
    scale = n_out / n_in

    def kern(x):
        x = np.abs(x); a = -0.5
        return np.where(x <= 1, (a + 2) * x**3 - (a + 3) * x**2 + 1,
                        np.where(x < 2, a * x**3 - 5 * a * x**2 + 8 * a * x - 4 * a, 0.0))

    sample_f = (np.arange(n_out, dtype=np.float64) + 0.5) / scale - 0.5
    x = np.abs(sample_f[None, :] - np.arange(n_in, dtype=np.float64)[:, None])
    wts = kern(x)
    tot = wts.sum(axis=0, keepdims=True)
    wts = np.where(np.abs(tot) > 1000 * np.finfo(np.float32).eps, wts / tot, 0)
    wts = np.where(((sample_f >= -0.5) & (sample_f <= n_in - 0.5))[None, :], wts, 0)
    return wts.astype(np.float32)  # [n_in, n_out]


# ------------------------------------------------------------------ device
_NC_CACHE = [None]


def build_nc():
    import concourse.bacc as bacc
    import concourse.mybir as mybir
    from concourse.tile import TileContext
    fp32, fp16, i16 = mybir.dt.float32, mybir.dt.float16, mybir.dt.int16
    AL = mybir.AluOpType

    nc = bacc.Bacc("TRN2", target_bir_lowering=False, debug=False)

    # host-prepped image, even/odd pair arrays: [8 samples, 3c, 2eo, 224, 224]
    y16_d = nc.dram_tensor("y16", [ROUNDS, 8, C, 2, H, W], fp16, kind="ExternalInput")
    idxA_d = nc.dram_tensor("idxA", [2, ROUNDS, NBAND * NCH, 128, NI // 16], i16, kind="ExternalInput")
    idxB_d = nc.dram_tensor("idxB", [2, ROUNDS, NBAND * NCH, 128, NI // 16], i16, kind="ExternalInput")
    # weights, compact (per sample, not per channel): [w, r, ci, AB, 8, NI, 2]
    wt_d = nc.dram_tensor("wts", [2, ROUNDS, NBAND * NCH, 2, 8, NI, 2], fp16, kind="ExternalInput")
    rmov_d = nc.dram_tensor("rmov", [SPC, 112, 2, 2, W], fp16, kind="ExternalInput")
    out_d = nc.dram_tensor("outp", [SPC, C, H, W], fp32, kind="ExternalOutput")
    # DRAM staging for warp outputs
    w1_d = nc.dram_tensor("w1stage", [ROUNDS, 8, C, 2, H, W], fp16)
    w2_d = nc.dram_tensor("w2stage", [ROUNDS, 8, C, H, W], fp16)

    with TileContext(nc) as tc:
        with tc.tile_pool(name="bigp", bufs=1) as bigp, \
             tc.tile_pool(name="smp", bufs=1) as smp, \
             tc.tile_pool(name="psp", bufs=2, space="PSUM") as psp:

            bnd = bigp.tile([128, 2, BAND_ROWS, W], fp16, tag="bnd")
            nc.vector.memset(bnd[:], 0.0)
            ga = smp.tile([128, NI, 2], fp16, tag="ga")
            gb = smp.tile([128, NI, 2], fp16, tag="gb")
            nc.vector.memset(ga[:], 0.0)
            nc.vector.memset(gb[:], 0.0)
            wta = smp.tile([128, NI, 2], fp16, tag="wta")
            wtb = smp.tile([128, NI, 2], fp16, tag="wtb")
            nc.vector.memset(wta[:], 0.0)
            nc.vector.memset(wtb[:], 0.0)

            for r in range(ROUNDS):
                for w in range(2):
                    for b in range(NBAND):
                        r0 = band_r0(b)
                        # band load: dst partitions 16s+c (per c: partition step 16)
                        for c in range(C):
                            if w == 0:
                                src = y16_d[r, :, c, :, r0:r0 + BAND_ROWS, :]
                            else:
                                src = w1_d[r, :, c, :, r0:r0 + BAND_ROWS, :]
                            nc.sync.dma_start(
                                out=bnd[c::16, :, :, :], in_=src)
                        for ch in range(NCH):
                            ci = b * NCH + ch
                            ia = smp.tile([128, NI // 16], i16, tag="ia")
                            ib = smp.tile([128, NI // 16], i16, tag="ib")
                            nc.sync.dma_start(out=ia[:], in_=idxA_d[w, r, ci, :, :])
                            nc.sync.dma_start(out=ib[:], in_=idxB_d[w, r, ci, :, :])
                            for c in range(C):
                                nc.sync.dma_start(out=wta[c::16, :, :], in_=wt_d[w, r, ci, 0, :, :, :])
                                nc.sync.dma_start(out=wtb[c::16, :, :], in_=wt_d[w, r, ci, 1, :, :, :])
                            dat = bnd[:].rearrange("p a b c -> p (a b c)").rearrange("p (n d) -> p n d", d=2)
                            nc.gpsimd.ap_gather(ga[:, :, :], dat, ia[:, :],
                                                channels=128, num_elems=NE, d=2, num_idxs=NI)
                            nc.gpsimd.ap_gather(gb[:, :, :], dat, ib[:, :],
                                                channels=128, num_elems=NE, d=2, num_idxs=NI)
                            # blend
                            t1 = smp.tile([128, NI, 2], fp32, tag="t1")
                            t2 = smp.tile([128, NI, 2], fp32, tag="t2")
                            nc.vector.tensor_tensor(out=t1[:], in0=ga[:], in1=wta[:], op=AL.mult)
                            nc.vector.tensor_tensor(out=t2[:], in0=gb[:], in1=wtb[:], op=AL.mult)
                            s1 = smp.tile([128, NI], fp32, tag="s1")
                            s2 = smp.tile([128, NI], fp32, tag="s2")
                            nc.vector.tensor_tensor(out=s1[:], in0=t1[:, :, 0], in1=t1[:, :, 1], op=AL.add)
                            nc.vector.tensor_tensor(out=s2[:], in0=t2[:, :, 0], in1=t2[:, :, 1], op=AL.add)
                            res = smp.tile([128, NI], fp16, tag="res")
                            nc.vector.tensor_tensor(out=res[:], in0=s1[:], in1=s2[:], op=AL.add)
                            # stage to DRAM
                            rr = RB * b + CROWS * ch
                            for c in range(C):
                                rv = res[c::16, :].rearrange("p (a b) -> p a b", a=CROWS)
                                if w == 0:
                                    nc.sync.dma_start(out=w1_d[r, :, c, 0, rr:rr + CROWS, :], in_=rv)
                                    # odd-shifted copy (cols 1.. -> 0..)
                                    nc.sync.dma_start(
                                        out=w1_d[r, :, c, 1, rr:rr + CROWS, 0:W - 1],
                                        in_=rv[:, :, 1:])
                                else:
                                    nc.sync.dma_start(out=w2_d[r, :, c, rr:rr + CROWS, :], in_=rv)
                    tc.strict_bb_all_engine_barrier()

                # ---- resize per sample
                for s in range(8):
                    sg = r * 8 + s
                    yrs = smp.tile([112, 2, C, W], fp16, tag="yrs")
                    # rows layout: partition p, half u: row h = u*112+p
                    for c in range(C):
                        nc.sync.dma_start(
                            out=yrs[:, :, c, :],
                            in_=w2_d[r, s, c, :, :].rearrange("(u p) x -> p u x", u=2))
                    rmv = smp.tile([112, 2, 2, W], fp16, tag="rmv")
                    nc.sync.dma_start(out=rmv[:], in_=rmov_d[sg, :, :, :, :])
                    for c in range(C):
                        t1t = smp.tile([112, 2, 224], fp16, tag="t1t")
                        for mh in range(2):
                            acc = psp.tile([112, W], fp32, tag="acc")
                            for kh in range(2):
                                nc.tensor.matmul(
                                    acc[:],
                                    yrs[:, kh, c, mh * 112:(mh + 1) * 112],
                                    rmv[:, 0, kh, :],
                                    start=(kh == 0), stop=(kh == 1))
                            nc.scalar.copy(out=t1t[:, mh, :], in_=acc[:])
                        ost = smp.tile([112, W], fp32, tag="ost")
                        for mh2 in range(2):
                            acc2 = psp.tile([112, W], fp32, tag="acc2")
                            for kh2 in range(2):
                                nc.tensor.matmul(
                                    acc2[:],
                                    t1t[:, kh2, mh2 * 112:(mh2 + 1) * 112],
                                    rmv[:, 1, kh2, :],
                                    start=(kh2 == 0), stop=(kh2 == 1))
                            nc.scalar.copy(out=ost[:], in_=acc2[:])
                            nc.sync.dma_start(
                                out=out_d[sg, c, mh2 * 112:(mh2 + 1) * 112, :], in_=ost[:])
    nc.compile()
    return nc


# ------------------------------------------------------------------ driver
def _host_pack(inputs):
    x = np.asarray(inputs['x'], np.float32)
    noise = np.asarray(inputs['noise'], np.float32)
    bright = np.asarray(inputs['brightness'], np.float32)
    flip = np.asarray(inputs['flip_mask'], np.int32)
    ep = np.asarray(inputs['ep_raw'], np.int32)
    ang = np.asarray(inputs['angles'], np.int32)
    cij = np.asarray(inputs['crop_ij'], np.int32)

    # prep: flip + noise + brightness (exact fp32 order as reference), cast fp16
    xf = np.where(flip[:, None, None, None] > 0, x[..., ::-1], x)
    y = (xf + np.float32(0.625) * noise) * (np.float32(0.85) + np.float32(0.30) * bright)[:, None, None, None]
    y16 = y.astype(np.float16)

    R190 = bicubic_weight_mat(CROP, H)     # [190, 224]

    per_core = []
    for core in range(NCORES):
        sl = slice(core * SPC, (core + 1) * SPC)
        ys = y16[sl]                       # [32, 3, 224, 224]
        # even/odd pair arrays
        yy = np.zeros((ROUNDS, 8, C, 2, H, W), np.float16)
        yy[:, :, :, 0] = ys.reshape(ROUNDS, 8, C, H, W)
        yy[:, :, :, 1, :, :W - 1] = ys.reshape(ROUNDS, 8, C, H, W)[..., 1:]

        idxA = np.zeros((2, ROUNDS, NBAND * NCH, 128, NI // 16), np.int16)
        idxB = np.zeros_like(idxA)
        wts = np.zeros((2, ROUNDS, NBAND * NCH, 2, 8, NI, 2), np.float16)
        for rr in range(ROUNDS):
            for s in range(8):
                sg = core * SPC + rr * 8 + s
                for w in range(2):
                    if w == 0:
                        sx, sy = persp_grid(ep[sg])
                    else:
                        sx, sy = rot_grid(ang[sg])
                    iA, iB, wA, wB = pack_warp(sx, sy)
                    for ci in range(NBAND * NCH):
                        idxA[w, rr, ci, 16 * s:16 * s + 16, :] = wrap16(iA[ci])
                        idxB[w, rr, ci, 16 * s:16 * s + 16, :] = wrap16(iB[ci])
                        wts[w, rr, ci, 0, s] = wA[ci]
                        wts[w, rr, ci, 1, s] = wB[ci]

        rmov = np.zeros((SPC, 112, 2, 2, W), np.float16)
        for si in range(SPC):
            sg = core * SPC + si
            i0, j0 = int(cij[sg, 0]), int(cij[sg, 1])
            Rh = np.zeros((H, H), np.float32)
            Rw = np.zeros((H, H), np.float32)
            Rh[i0:i0 + CROP, :] = R190
            Rw[j0:j0 + CROP, :] = R190
            # [p, hw, kh, W]: stationary k-chunk kh covers rows kh*112+p
            rmov[si, :, 0, :, :] = Rh.reshape(2, 112, W).transpose(1, 0, 2).astype(np.float16)
            rmov[si, :, 1, :, :] = Rw.reshape(2, 112, W).transpose(1, 0, 2).astype(np.float16)

        per_core.append({
            "y16": yy, "idxA": idxA, "idxB": idxB, "wts": wts, "rmov": rmov,
        })
    return per_core


def _axon_shim():
    """Make trace=True work under axon (missing antenv.axon_hooks in image)
    and stub the artifact upload (zero-egress container)."""
    import types
    try:
        import antenv.axon_hooks  # noqa
    except ImportError:
        mod = types.ModuleType('antenv.axon_hooks')
        mod._hook = None
        mod.set_axon_ntff_profile_hook = lambda h: setattr(mod, '_hook', h)
        mod.get_axon_ntff_profile_hook = lambda: mod._hook
        sys.modules['antenv.axon_hooks'] = mod
        import antenv
        antenv.axon_hooks = mod
    from antenv.axon_hooks import get_axon_ntff_profile_hook, set_axon_ntff_profile_hook
    if get_axon_ntff_profile_hook() is None:
        try:
            from trn_agent_boot.trn_boot import _ntff_profile_via_ctypes
            set_axon_ntff_profile_hook(_ntff_profile_via_ctypes('/opt/axon/libaxon_pjrt.so'))
        except Exception:
            pass
    from concourse import bass_utils
    bass_utils.upload_artifacts = lambda tmpdir: f"local://{tmpdir}"


def kernel(**inputs):
    _axon_shim()
    from concourse import bass_utils

    per_core = _host_pack(inputs)
    if _NC_CACHE[0] is None:
        _NC_CACHE[0] = build_nc()
    nc = _NC_CACHE[0]

    import os
    trace = bool(int(os.environ.get("KERNEL_TRACE", "0")))
    res = bass_utils.run_bass_kernel_spmd(
        nc, per_core, list(range(NCORES)), trace=trace)
    if trace and res.exec_time_ns is not None:
        print(f"HW exec time: {res.exec_time_ns} ns")
        kernel.last_exec_ns = res.exec_time_ns
    out = np.concatenate([res.results[i]["outp"] for i in range(NCORES)], axis=0)
    return out.astype(np.float32)



# revision 2
# speedup vs baseline: 1.0398x; 1.0398x over previous
"""Trainium2 Bass kernel for nn_DataAugmentation_46823733461007 (8 NeuronCores).

Reference pipeline per sample: hflip, +0.625*noise, *brightness, perspective
warp (bilinear), rotation warp (bilinear), 190x190 crop, bicubic resize to
224x224.

Mapping:
  host  - folds flip/noise/brightness into a fp16 image (even + odd-shifted
          pair arrays), solves the perspective homography, and precomputes for
          both warps per-pixel pair-gather indices + premultiplied blend
          weights (exact decomposition of the reference bilinear incl.
          zero-fill validity), plus per-sample crop-fused bicubic matrices.
  device- per warp: row-band loads -> gpsimd ap_gather (d=2 pairs; 8 samples
          per instruction via the 16-partition index groups) -> DVE blend ->
          staged to DRAM; then two PE matmuls per channel apply
          crop+bicubic-resize; data-parallel over 8 cores, 32 samples each.
"""
import sys
sys.path.insert(0, '/opt/trn_rl_repo')
import numpy as np

B, C, H, W = 256, 3, 224, 224
CROP = 190
NCORES = 8
SPC = B // NCORES          # 32 samples per core
ROUNDS = SPC // 8          # 4 rounds x 8 samples
NBAND = 7
RB = H // NBAND            # 32 output rows per band
BAND_ROWS = RB + 75        # 107 source rows held per band
NCH = 4                    # chunks per band
CROWS = RB // NCH          # 8 rows per chunk
NI = CROWS * W             # 1792 indices per chunk per group
NPAIR = W // 2             # 112 pairs per row per parity
NE = 2 * BAND_ROWS * NPAIR # ap_gather num_elems = 23968


# ------------------------------------------------------------------- host
_XG, _YG = np.meshgrid(np.arange(W, dtype=np.float32) + 0.5,
                       np.arange(H, dtype=np.float32) + 0.5, indexing='xy')


def _persp_coeffs(ep_raw_s):
    offs = np.array([[0., 0.], [195., 0.], [195., 195.], [0., 195.]], np.float32)
    start = np.array([[0., 0.], [223., 0.], [223., 223.], [0., 223.]], np.float32)
    end = ep_raw_s.astype(np.float32) + offs
    ex, ey = end[:, 0], end[:, 1]
    sx, sy = start[:, 0], start[:, 1]
    o = np.ones(4, np.float32); z = np.zeros(4, np.float32)
    r1 = np.stack([ex, ey, o, z, z, z, -sx * ex, -sx * ey], axis=-1)
    r2 = np.stack([z, z, z, ex, ey, o, -sy * ex, -sy * ey], axis=-1)
    A = np.concatenate([r1, r2], axis=0).astype(np.float32)
    b = np.concatenate([sx, sy], axis=0).astype(np.float32)
    return np.linalg.solve(A, b).astype(np.float32)


def persp_grid(ep_raw_s):
    c = _persp_coeffs(ep_raw_s)
    a, b, cc, d, e, f, g, h = [np.float32(c[i]) for i in range(8)]
    den = g * _XG + h * _YG + np.float32(1.0)
    sx = (a * _XG + b * _YG + cc) / den - np.float32(0.5)
    sy = (d * _XG + e * _YG + f) / den - np.float32(0.5)
    return sx.astype(np.float32), sy.astype(np.float32)


def rot_grid(angle):
    th = (np.float32(angle) - np.float32(16.0)) * np.float32(np.pi / 180.0)
    cos = np.float32(np.cos(th)); sin = np.float32(np.sin(th))
    cx = np.float32((W - 1) / 2.0); cy = np.float32((H - 1) / 2.0)
    dx = (_XG - np.float32(0.5)) - cx
    dy = (_YG - np.float32(0.5)) - cy
    rx = (cos * dx + sin * dy + cx).astype(np.float32)
    ry = (-sin * dx + cos * dy + cy).astype(np.float32)
    return rx, ry


def warp_fields(sx, sy):
    """Exact pair-gather decomposition of the reference `_bilinear`."""
    x0 = np.floor(sx); y0 = np.floor(sy)
    wx = (sx - x0).astype(np.float32); wy = (sy - y0).astype(np.float32)
    x0i = x0.astype(np.int64); y0i = y0.astype(np.int64)

    vx0 = ((x0i >= 0) & (x0i < W)).astype(np.float32)
    vx1 = (x0i + 1 < W).astype(np.float32) * (x0i + 1 >= 0)
    vy0 = ((y0i >= 0) & (y0i < H)).astype(np.float32)
    vy1 = (y0i + 1 < H).astype(np.float32) * (y0i + 1 >= 0)

    y0c = np.clip(y0i, 0, H - 1)
    y1c = np.clip(y0i + 1, 0, H - 1)
    x0c = np.clip(x0i, 0, W - 1)
    x1c = np.clip(x0i + 1, 0, W - 1)
    px = np.clip(x0i, 0, W - 2)

    w_e0 = np.where(x0c == px, (1 - wx) * vx0, 0.0).astype(np.float32) \
         + np.where(x1c == px, wx * vx1, 0.0).astype(np.float32)
    w_e1 = np.where(x0c == px + 1, (1 - wx) * vx0, 0.0).astype(np.float32) \
         + np.where(x1c == px + 1, wx * vx1, 0.0).astype(np.float32)

    wy0 = ((1 - wy) * vy0).astype(np.float32)
    wy1 = (wy * vy1).astype(np.float32)
    return y0c, y1c, px, w_e0, w_e1, wy0, wy1


def band_r0(b):
    return min(max(RB * b - 37, 0), H - BAND_ROWS)


def pack_warp(sx, sy):
    """-> idxA, idxB: [NBAND*NCH, NI] int16 (band-window-relative pair idx);
       wtA, wtB: [NBAND*NCH, NI, 2] fp16 premultiplied blend weights."""
    y0c, y1c, px, w_e0, w_e1, wy0, wy1 = warp_fields(sx, sy)
    eo = (px & 1).astype(np.int64)
    pr = (px >> 1).astype(np.int64)
    iA = np.empty((H, W), np.int64)
    iB = np.empty((H, W), np.int64)
    for b in range(NBAND):
        r0 = band_r0(b)
        rs = slice(RB * b, RB * (b + 1))
        iA[rs] = eo[rs] * (BAND_ROWS * NPAIR) + (y0c[rs] - r0) * NPAIR + pr[rs]
        iB[rs] = eo[rs] * (BAND_ROWS * NPAIR) + (y1c[rs] - r0) * NPAIR + pr[rs]
    assert iA.min() >= 0 and iA.max() < NE and iB.min() >= 0 and iB.max() < NE
    idxA = iA.reshape(NBAND * NCH, NI).astype(np.int16)
    idxB = iB.reshape(NBAND * NCH, NI).astype(np.int16)
    wtA = np.stack([wy0 * w_e0, wy0 * w_e1], axis=-1).reshape(NBAND * NCH, NI, 2).astype(np.float16)
    wtB = np.stack([wy1 * w_e0, wy1 * w_e1], axis=-1).reshape(NBAND * NCH, NI, 2).astype(np.float16)
    return idxA, idxB, wtA, wtB


def wrap16(u):
    """[NI] -> [16, NI//16] wrapped layout for one group."""
    return u.reshape(NI // 16, 16).T


def bicubic_weight_mat(n_in, n_out):
    scale = n_out / n_in

    def kern(x):
        x = np.abs(x); a = -0.5
        return np.where(x <= 1, (a + 2) * x**3 - (a + 3) * x**2 + 1,
                        np.where(x < 2, a * x**3 - 5 * a * x**2 + 8 * a * x - 4 * a, 0.0))

    sample_f = (np.arange(n_out, dtype=np.float64) + 0.5) / scale - 0.5
    x = np.abs(sample_f[None, :] - np.arange(n_in, dtype=np.float64)[:, None])
    wts = kern(x)
    tot = wts.sum(axis=0, keepdims=True)
    wts = np.where(np.abs(tot) > 1000 * np.finfo(np.float32).eps, wts / tot, 0)
    wts = np.where(((sample_f >= -0.5) & (sample_f <= n_in - 0.5))[None, :], wts, 0)
    return wts.astype(np.float32)  # [n_in, n_out]


# ------------------------------------------------------------------ device
_NC_CACHE = [None]


def build_nc():
    import concourse.bacc as bacc
    import concourse.mybir as mybir
    from concourse.tile import TileContext
    fp32, fp16, i16 = mybir.dt.float32, mybir.dt.float16, mybir.dt.int16
    AL = mybir.AluOpType

    nc = bacc.Bacc("TRN2", target_bir_lowering=False, debug=False)

    # host-prepped image, even/odd pair arrays: [8 samples, 3c, 2eo, 224, 224]
    y16_d = nc.dram_tensor("y16", [ROUNDS, 8, C, 2, H, W], fp16, kind="ExternalInput")
    idxA_d = nc.dram_tensor("idxA", [2, ROUNDS, NBAND * NCH, 128, NI // 16], i16, kind="ExternalInput")
    idxB_d = nc.dram_tensor("idxB", [2, ROUNDS, NBAND * NCH, 128, NI // 16], i16, kind="ExternalInput")
    # weights, compact (per sample, not per channel): [w, r, ci, AB, 8, NI, 2]
    wt_d = nc.dram_tensor("wts", [2, ROUNDS, NBAND * NCH, 2, 8, NI, 2], fp16, kind="ExternalInput")
    rmov_d = nc.dram_tensor("rmov", [SPC, 112, 2, 2, W], fp16, kind="ExternalInput")
    out_d = nc.dram_tensor("outp", [SPC, C, H, W], fp32, kind="ExternalOutput")
    # DRAM staging for warp outputs
    w1_d = nc.dram_tensor("w1stage", [ROUNDS, 8, C, 2, H, W], fp16)
    w2_d = nc.dram_tensor("w2stage", [ROUNDS, 8, C, H, W], fp16)

    with TileContext(nc) as tc:
        with tc.tile_pool(name="bigp", bufs=1) as bigp, \
             tc.tile_pool(name="smp", bufs=2) as smp, \
             tc.tile_pool(name="rsz", bufs=2) as rsz, \
             tc.tile_pool(name="psp", bufs=2, space="PSUM") as psp:

            bnd = bigp.tile([128, 2, BAND_ROWS, W], fp16, tag="bnd")
            nc.vector.memset(bnd[:], 0.0)

            for r in range(ROUNDS):
                for w in range(2):
                    for b in range(NBAND):
                        r0 = band_r0(b)
                        # band load: dst partitions 16s+c (per c: partition step 16)
                        for c in range(C):
                            if w == 0:
                                src = y16_d[r, :, c, :, r0:r0 + BAND_ROWS, :]
                            else:
                                src = w1_d[r, :, c, :, r0:r0 + BAND_ROWS, :]
                            nc.sync.dma_start(
                                out=bnd[c::16, :, :, :], in_=src)
                        for ch in range(NCH):
                            ci = b * NCH + ch
                            ia = smp.tile([128, NI // 16], i16, tag="ia")
                            ib = smp.tile([128, NI // 16], i16, tag="ib")
                            nc.sync.dma_start(out=ia[:], in_=idxA_d[w, r, ci, :, :])
                            nc.sync.dma_start(out=ib[:], in_=idxB_d[w, r, ci, :, :])
                            wta = smp.tile([128, NI, 2], fp16, tag="wta")
                            wtb = smp.tile([128, NI, 2], fp16, tag="wtb")
                            for c in range(C):
                                nc.sync.dma_start(out=wta[c::16, :, :], in_=wt_d[w, r, ci, 0, :, :, :])
                                nc.sync.dma_start(out=wtb[c::16, :, :], in_=wt_d[w, r, ci, 1, :, :, :])
                            ga = smp.tile([128, NI, 2], fp16, tag="ga")
                            gb = smp.tile([128, NI, 2], fp16, tag="gb")
                            dat = bnd[:].rearrange("p a b c -> p (a b c)").rearrange("p (n d) -> p n d", d=2)
                            nc.gpsimd.ap_gather(ga[:, :, :], dat, ia[:, :],
                                                channels=128, num_elems=NE, d=2, num_idxs=NI)
                            nc.gpsimd.ap_gather(gb[:, :, :], dat, ib[:, :],
                                                channels=128, num_elems=NE, d=2, num_idxs=NI)
                            # blend (fp16, in-place products)
                            nc.vector.tensor_tensor(out=ga[:], in0=ga[:], in1=wta[:], op=AL.mult)
                            nc.vector.tensor_tensor(out=gb[:], in0=gb[:], in1=wtb[:], op=AL.mult)
                            s1 = smp.tile([128, NI], fp16, tag="s1")
                            s2 = smp.tile([128, NI], fp16, tag="s2")
                            nc.vector.tensor_tensor(out=s1[:], in0=ga[:, :, 0], in1=ga[:, :, 1], op=AL.add)
                            nc.vector.tensor_tensor(out=s2[:], in0=gb[:, :, 0], in1=gb[:, :, 1], op=AL.add)
                            res = smp.tile([128, NI], fp16, tag="res")
                            nc.vector.tensor_tensor(out=res[:], in0=s1[:], in1=s2[:], op=AL.add)
                            # stage to DRAM (dispatch on Scalar engine queue)
                            rr = RB * b + CROWS * ch
                            for c in range(C):
                                rv = res[c::16, :].rearrange("p (a b) -> p a b", a=CROWS)
                                if w == 0:
                                    nc.scalar.dma_start(out=w1_d[r, :, c, 0, rr:rr + CROWS, :], in_=rv)
                                    # odd-shifted copy (cols 1.. -> 0..)
                                    nc.scalar.dma_start(
                                        out=w1_d[r, :, c, 1, rr:rr + CROWS, 0:W - 1],
                                        in_=rv[:, :, 1:])
                                else:
                                    nc.scalar.dma_start(out=w2_d[r, :, c, rr:rr + CROWS, :], in_=rv)

                # ---- resize per sample
                for s in range(8):
                    sg = r * 8 + s
                    yrs = rsz.tile([112, 2, C, W], fp16, tag="yrs")
                    # rows layout: partition p, half u: row h = u*112+p
                    for c in range(C):
                        nc.sync.dma_start(
                            out=yrs[:, :, c, :],
                            in_=w2_d[r, s, c, :, :].rearrange("(u p) x -> p u x", u=2))
                    rmv = rsz.tile([112, 2, 2, W], fp16, tag="rmv")
                    nc.sync.dma_start(out=rmv[:], in_=rmov_d[sg, :, :, :, :])
                    for c in range(C):
                        t1t = rsz.tile([112, 2, 224], fp16, tag="t1t")
                        for mh in range(2):
                            acc = psp.tile([112, W], fp32, tag="acc")
                            for kh in range(2):
                                nc.tensor.matmul(
                                    acc[:],
                                    yrs[:, kh, c, mh * 112:(mh + 1) * 112],
                                    rmv[:, 0, kh, :],
                                    start=(kh == 0), stop=(kh == 1))
                            nc.scalar.copy(out=t1t[:, mh, :], in_=acc[:])
                        ost = rsz.tile([112, W], fp32, tag="ost")
                        for mh2 in range(2):
                            acc2 = psp.tile([112, W], fp32, tag="acc2")
                            for kh2 in range(2):
                                nc.tensor.matmul(
                                    acc2[:],
                                    t1t[:, kh2, mh2 * 112:(mh2 + 1) * 112],
                                    rmv[:, 1, kh2, :],
                                    start=(kh2 == 0), stop=(kh2 == 1))
                            nc.scalar.copy(out=ost[:], in_=acc2[:])
                            nc.sync.dma_start(
                                out=out_d[sg, c, mh2 * 112:(mh2 + 1) * 112, :], in_=ost[:])
    nc.compile()
    return nc


# ------------------------------------------------------------------ driver
def _host_pack(inputs):
    x = np.asarray(inputs['x'], np.float32)
    noise = np.asarray(inputs['noise'], np.float32)
    bright = np.asarray(inputs['brightness'], np.float32)
    flip = np.asarray(inputs['flip_mask'], np.int32)
    ep = np.asarray(inputs['ep_raw'], np.int32)
    ang = np.asarray(inputs['angles'], np.int32)
    cij = np.asarray(inputs['crop_ij'], np.int32)

    # prep: flip + noise + brightness (exact fp32 order as reference), cast fp16
    xf = np.where(flip[:, None, None, None] > 0, x[..., ::-1], x)
    y = (xf + np.float32(0.625) * noise) * (np.float32(0.85) + np.float32(0.30) * bright)[:, None, None, None]
    y16 = y.astype(np.float16)

    R190 = bicubic_weight_mat(CROP, H)     # [190, 224]

    per_core = []
    for core in range(NCORES):
        sl = slice(core * SPC, (core + 1) * SPC)
        ys = y16[sl]                       # [32, 3, 224, 224]
        # even/odd pair arrays
        yy = np.zeros((ROUNDS, 8, C, 2, H, W), np.float16)
        yy[:, :, :, 0] = ys.reshape(ROUNDS, 8, C, H, W)
        yy[:, :, :, 1, :, :W - 1] = ys.reshape(ROUNDS, 8, C, H, W)[..., 1:]

        idxA = np.zeros((2, ROUNDS, NBAND * NCH, 128, NI // 16), np.int16)
        idxB = np.zeros_like(idxA)
        wts = np.zeros((2, ROUNDS, NBAND * NCH, 2, 8, NI, 2), np.float16)
        for rr in range(ROUNDS):
            for s in range(8):
                sg = core * SPC + rr * 8 + s
                for w in range(2):
                    if w == 0:
                        sx, sy = persp_grid(ep[sg])
                    else:
                        sx, sy = rot_grid(ang[sg])
                    iA, iB, wA, wB = pack_warp(sx, sy)
                    for ci in range(NBAND * NCH):
                        idxA[w, rr, ci, 16 * s:16 * s + 16, :] = wrap16(iA[ci])
                        idxB[w, rr, ci, 16 * s:16 * s + 16, :] = wrap16(iB[ci])
                        wts[w, rr, ci, 0, s] = wA[ci]
                        wts[w, rr, ci, 1, s] = wB[ci]

        rmov = np.zeros((SPC, 112, 2, 2, W), np.float16)
        for si in range(SPC):
            sg = core * SPC + si
            i0, j0 = int(cij[sg, 0]), int(cij[sg, 1])
            Rh = np.zeros((H, H), np.float32)
            Rw = np.zeros((H, H), np.float32)
            Rh[i0:i0 + CROP, :] = R190
            Rw[j0:j0 + CROP, :] = R190
            # [p, hw, kh, W]: stationary k-chunk kh covers rows kh*112+p
            rmov[si, :, 0, :, :] = Rh.reshape(2, 112, W).transpose(1, 0, 2).astype(np.float16)
            rmov[si, :, 1, :, :] = Rw.reshape(2, 112, W).transpose(1, 0, 2).astype(np.float16)

        per_core.append({
            "y16": yy, "idxA": idxA, "idxB": idxB, "wts": wts, "rmov": rmov,
        })
    return per_core


def _axon_shim():
    """Make trace=True work under axon (missing antenv.axon_hooks in image)
    and stub the artifact upload (zero-egress container)."""
    import types
    try:
        import antenv.axon_hooks  # noqa
    except ImportError:
        mod = types.ModuleType('antenv.axon_hooks')
        mod._hook = None
        mod.set_axon_ntff_profile_hook = lambda h: setattr(mod, '_hook', h)
        mod.get_axon_ntff_profile_hook = lambda: mod._hook
        sys.modules['antenv.axon_hooks'] = mod
        import antenv
        antenv.axon_hooks = mod
    from antenv.axon_hooks import get_axon_ntff_profile_hook, set_axon_ntff_profile_hook
    if get_axon_ntff_profile_hook() is None:
        try:
            from trn_agent_boot.trn_boot import _ntff_profile_via_ctypes
            set_axon_ntff_profile_hook(_ntff_profile_via_ctypes('/opt/axon/libaxon_pjrt.so'))
        except Exception:
            pass
    from concourse import bass_utils
    bass_utils.upload_artifacts = lambda tmpdir: f"local://{tmpdir}"


def kernel(**inputs):
    _axon_shim()
    from concourse import bass_utils

    per_core = _host_pack(inputs)
    if _NC_CACHE[0] is None:
        _NC_CACHE[0] = build_nc()
    nc = _NC_CACHE[0]

    import os
    trace = bool(int(os.environ.get("KERNEL_TRACE", "0")))
    res = bass_utils.run_bass_kernel_spmd(
        nc, per_core, list(range(NCORES)), trace=trace)
    if trace and res.exec_time_ns is not None:
        print(f"HW exec time: {res.exec_time_ns} ns")
        kernel.last_exec_ns = res.exec_time_ns
    out = np.concatenate([res.results[i]["outp"] for i in range(NCORES)], axis=0)
    return out.astype(np.float32)



# revision 12
# speedup vs baseline: 1.7567x; 1.6895x over previous
"""Trainium2 Bass kernel for nn_DataAugmentation_46823733461007 (8 NeuronCores).

Reference pipeline per sample: hflip, +0.625*noise, *brightness, perspective
warp (bilinear), rotation warp (bilinear), 190x190 crop, bicubic resize to
224x224.

Mapping:
  host  - folds flip/noise/brightness into a fp16 image (even + odd-shifted
          pair arrays), solves the perspective homography, and precomputes for
          both warps per-pixel pair-gather indices + premultiplied blend
          weights (exact decomposition of the reference bilinear incl.
          zero-fill validity), plus per-sample crop-fused bicubic matrices.
  device- per warp: row-band loads -> gpsimd ap_gather (d=2 pairs; 8 samples
          per instruction via the 16-partition index groups) -> DVE blend ->
          staged to DRAM; then two PE matmuls per channel apply
          crop+bicubic-resize; data-parallel over 8 cores, 32 samples each.
"""
import sys
sys.path.insert(0, '/opt/trn_rl_repo')
import numpy as np

B, C, H, W = 256, 3, 224, 224
CROP = 190
NCORES = 8
SPC = B // NCORES          # 32 samples per core
ROUNDS = SPC // 8          # 4 rounds x 8 samples
NBAND = 7
RB = H // NBAND            # 32 output rows per band
BAND_ROWS = RB + 75        # 107 source rows held per band
BRV = BAND_ROWS + 1        # 108 slots (vert-lane window: v=0 rows r0-1.., v=1 rows r0..)
NCH = 4                    # chunks per band
CROWS = RB // NCH          # 8 rows per chunk
NI = CROWS * W             # 1792 indices per chunk per group
NPAIR = W // 2             # 112 pairs per row per parity
NEV = 2 * BRV * NPAIR      # ap_gather num_elems = 24192


# ------------------------------------------------------------------- host
_XG, _YG = np.meshgrid(np.arange(W, dtype=np.float32) + 0.5,
                       np.arange(H, dtype=np.float32) + 0.5, indexing='xy')


def _persp_coeffs(ep_raw_s):
    offs = np.array([[0., 0.], [195., 0.], [195., 195.], [0., 195.]], np.float32)
    start = np.array([[0., 0.], [223., 0.], [223., 223.], [0., 223.]], np.float32)
    end = ep_raw_s.astype(np.float32) + offs
    ex, ey = end[:, 0], end[:, 1]
    sx, sy = start[:, 0], start[:, 1]
    o = np.ones(4, np.float32); z = np.zeros(4, np.float32)
    r1 = np.stack([ex, ey, o, z, z, z, -sx * ex, -sx * ey], axis=-1)
    r2 = np.stack([z, z, z, ex, ey, o, -sy * ex, -sy * ey], axis=-1)
    A = np.concatenate([r1, r2], axis=0).astype(np.float32)
    b = np.concatenate([sx, sy], axis=0).astype(np.float32)
    return np.linalg.solve(A, b).astype(np.float32)


def persp_grid(ep_raw_s):
    c = _persp_coeffs(ep_raw_s)
    a, b, cc, d, e, f, g, h = [np.float32(c[i]) for i in range(8)]
    den = g * _XG + h * _YG + np.float32(1.0)
    sx = (a * _XG + b * _YG + cc) / den - np.float32(0.5)
    sy = (d * _XG + e * _YG + f) / den - np.float32(0.5)
    return sx.astype(np.float32), sy.astype(np.float32)


def rot_grid(angle):
    th = (np.float32(angle) - np.float32(16.0)) * np.float32(np.pi / 180.0)
    cos = np.float32(np.cos(th)); sin = np.float32(np.sin(th))
    cx = np.float32((W - 1) / 2.0); cy = np.float32((H - 1) / 2.0)
    dx = (_XG - np.float32(0.5)) - cx
    dy = (_YG - np.float32(0.5)) - cy
    rx = (cos * dx + sin * dy + cx).astype(np.float32)
    ry = (-sin * dx + cos * dy + cy).astype(np.float32)
    return rx, ry


def warp_fields(sx, sy):
    """Exact pair-gather decomposition of the reference `_bilinear`."""
    x0 = np.floor(sx); y0 = np.floor(sy)
    wx = (sx - x0).astype(np.float32); wy = (sy - y0).astype(np.float32)
    x0i = x0.astype(np.int64); y0i = y0.astype(np.int64)

    vx0 = ((x0i >= 0) & (x0i < W)).astype(np.float32)
    vx1 = (x0i + 1 < W).astype(np.float32) * (x0i + 1 >= 0)
    vy0 = ((y0i >= 0) & (y0i < H)).astype(np.float32)
    vy1 = (y0i + 1 < H).astype(np.float32) * (y0i + 1 >= 0)

    y0c = np.clip(y0i, 0, H - 1)
    y1c = np.clip(y0i + 1, 0, H - 1)
    x0c = np.clip(x0i, 0, W - 1)
    x1c = np.clip(x0i + 1, 0, W - 1)
    px = np.clip(x0i, 0, W - 2)

    w_e0 = np.where(x0c == px, (1 - wx) * vx0, 0.0).astype(np.float32) \
         + np.where(x1c == px, wx * vx1, 0.0).astype(np.float32)
    w_e1 = np.where(x0c == px + 1, (1 - wx) * vx0, 0.0).astype(np.float32) \
         + np.where(x1c == px + 1, wx * vx1, 0.0).astype(np.float32)

    wy0 = ((1 - wy) * vy0).astype(np.float32)
    wy1 = (wy * vy1).astype(np.float32)
    return y0i, px, w_e0, w_e1, wy0, wy1


def band_r0(b):
    return min(max(RB * b - 37, 0), H - BAND_ROWS)


def pack_warp(sx, sy):
    """-> idxC: [NBAND*NCH, NI] int16 (vert-lane shared idx: v=0 lane holds
       row k-1+r0, v=1 lane row k+r0 at slot k);
       wtA, wtB: [NBAND*NCH, NI, 2] fp16 premultiplied blend weights."""
    y0i, px, w_e0, w_e1, wy0, wy1 = warp_fields(sx, sy)
    eo = (px & 1).astype(np.int64)
    pr = (px >> 1).astype(np.int64)
    nz = (wy0 != 0) | (wy1 != 0)
    iC = np.zeros((H, W), np.int64)
    for b in range(NBAND):
        r0 = band_r0(b)
        rs = slice(RB * b, RB * (b + 1))
        k = np.where(nz[rs], y0i[rs] + 1 - r0, 0)
        assert k.min() >= 0 and k.max() < BRV, (k.min(), k.max())
        iC[rs] = eo[rs] * (BRV * NPAIR) + k * NPAIR + pr[rs]
    assert iC.min() >= 0 and iC.max() < NEV
    idxC = iC.reshape(NBAND * NCH, NI).astype(np.int16)
    wtA = np.stack([wy0 * w_e0, wy0 * w_e1], axis=-1).reshape(NBAND * NCH, NI, 2).astype(np.float16)
    wtB = np.stack([wy1 * w_e0, wy1 * w_e1], axis=-1).reshape(NBAND * NCH, NI, 2).astype(np.float16)
    return idxC, wtA, wtB


def wrap16(u):
    """[NI] -> [16, NI//16] wrapped layout for one group."""
    return u.reshape(NI // 16, 16).T


def bicubic_weight_mat(n_in, n_out):
    scale = n_out / n_in

    def kern(x):
        x = np.abs(x); a = -0.5
        return np.where(x <= 1, (a + 2) * x**3 - (a + 3) * x**2 + 1,
                        np.where(x < 2, a * x**3 - 5 * a * x**2 + 8 * a * x - 4 * a, 0.0))

    sample_f = (np.arange(n_out, dtype=np.float64) + 0.5) / scale - 0.5
    x = np.abs(sample_f[None, :] - np.arange(n_in, dtype=np.float64)[:, None])
    wts = kern(x)
    tot = wts.sum(axis=0, keepdims=True)
    wts = np.where(np.abs(tot) > 1000 * np.finfo(np.float32).eps, wts / tot, 0)
    wts = np.where(((sample_f >= -0.5) & (sample_f <= n_in - 0.5))[None, :], wts, 0)
    return wts.astype(np.float32)  # [n_in, n_out]


# ------------------------------------------------------------------ device
_NC_CACHE = [None]


def build_nc():
    import concourse.bacc as bacc
    import concourse.mybir as mybir
    from concourse.tile import TileContext
    fp32, fp16, i16 = mybir.dt.float32, mybir.dt.float16, mybir.dt.int16
    AL = mybir.AluOpType

    nc = bacc.Bacc("TRN2", target_bir_lowering=False, debug=False)

    # host-prepped image, even/odd pair arrays: [8 samples, 3c, 2eo, 224, 224]
    y16_d = nc.dram_tensor("y16", [ROUNDS, 8, C, 2, H, W], fp16, kind="ExternalInput")
    idxC_d = nc.dram_tensor("idxC", [2, ROUNDS, NBAND * NCH, 128, NI // 16], i16, kind="ExternalInput")
    # weights, compact (per sample, not per channel): [w, r, ci, AB, 8, NI, 2]
    wt_d = nc.dram_tensor("wts", [2, ROUNDS, NBAND * NCH, 2, 8, NI, 2], fp16, kind="ExternalInput")
    rmov_d = nc.dram_tensor("rmov", [SPC, 112, 2, 2, W], fp16, kind="ExternalInput")
    out_d = nc.dram_tensor("outp", [SPC, C, H, W], fp32, kind="ExternalOutput")
    # DRAM staging for warp outputs
    w1_d = nc.dram_tensor("w1stage", [ROUNDS, 8, C, 2, H, W], fp16)
    w2_d = nc.dram_tensor("w2stage", [ROUNDS, 8, C, H, W], fp16)

    with TileContext(nc) as tc:
        with tc.tile_pool(name="bigp", bufs=1) as bigp, \
             tc.tile_pool(name="smp", bufs=2) as smp, \
             tc.tile_pool(name="rsz", bufs=2) as rsz, \
             tc.tile_pool(name="psp", bufs=2, space="PSUM") as psp:

            bnd = bigp.tile([128, 2, BRV, W], fp16, tag="bnd")
            nc.vector.memset(bnd[:], 0.0)

            for r in range(ROUNDS):
                for w in range(2):
                    for b in range(NBAND):
                        r0 = band_r0(b)
                        # band load: partition 16s+2c+v; lane v holds rows
                        # [r0-1+v, r0-1+v+BRV) at slots 0..BRV-1 (clipped)
                        for c in range(C):
                            for v in range(2):
                                lo_row = r0 - 1 + v
                                lo = max(lo_row, 0)
                                hi = min(lo_row + BRV, H)
                                if w == 0:
                                    src = y16_d[r, :, c, :, lo:hi, :]
                                else:
                                    src = w1_d[r, :, c, :, lo:hi, :]
                                nc.sync.dma_start(
                                    out=bnd[(2 * c + v)::16, :, lo - lo_row:hi - lo_row, :],
                                    in_=src)
                        for ch in range(NCH):
                            ci = b * NCH + ch
                            ix = smp.tile([128, NI // 16], i16, tag="ix")
                            nc.sync.dma_start(out=ix[:], in_=idxC_d[w, r, ci, :, :])
                            wt = smp.tile([128, NI, 2], fp16, tag="wt")
                            for c in range(C):
                                for v in range(2):
                                    nc.sync.dma_start(out=wt[(2 * c + v)::16, :, :],
                                                      in_=wt_d[w, r, ci, v, :, :, :])
                            ga = smp.tile([128, NI, 2], fp16, tag="ga")
                            dat = bnd[:].rearrange("p a b c -> p (a b c)").rearrange("p (n d) -> p n d", d=2)
                            nc.gpsimd.ap_gather(ga[:, :, :], dat, ix[:, :],
                                                channels=128, num_elems=NEV, d=2, num_idxs=NI)
                            # blend (fp16): products in place, pair-sum, then
                            # vert (lane) sum: SBUF->SBUF DMA shifts the v=1
                            # lanes onto the v=0 partitions, full-tile DVE add
                            nc.vector.tensor_tensor(out=ga[:], in0=ga[:], in1=wt[:], op=AL.mult)
                            res = smp.tile([128, NI], fp16, tag="res")
                            nc.vector.tensor_tensor(out=res[:], in0=ga[:, :, 0],
                                                    in1=ga[:, :, 1], op=AL.add)
                            tmp = smp.tile([128, NI], fp16, tag="tmp")
                            nc.sync.dma_start(out=tmp[0::2, :], in_=res[1::2, :])
                            fin = smp.tile([128, NI], fp16, tag="fin")
                            nc.vector.tensor_tensor(out=fin[:], in0=res[:], in1=tmp[:], op=AL.add)
                            # stage to DRAM (dispatch on Scalar engine queue)
                            rr = RB * b + CROWS * ch
                            for c in range(C):
                                rv = fin[(2 * c)::16, :].rearrange("p (a b) -> p a b", a=CROWS)
                                if w == 0:
                                    nc.scalar.dma_start(out=w1_d[r, :, c, 0, rr:rr + CROWS, :], in_=rv)
                                    # odd-shifted copy (cols 1.. -> 0..)
                                    nc.scalar.dma_start(
                                        out=w1_d[r, :, c, 1, rr:rr + CROWS, 0:W - 1],
                                        in_=rv[:, :, 1:])
                                else:
                                    nc.scalar.dma_start(out=w2_d[r, :, c, rr:rr + CROWS, :], in_=rv)

                # ---- resize per sample
                for s in range(8):
                    sg = r * 8 + s
                    yrs = rsz.tile([112, 2, C, W], fp16, tag="yrs")
                    # rows layout: partition p, half u: row h = u*112+p
                    for c in range(C):
                        nc.sync.dma_start(
                            out=yrs[:, :, c, :],
                            in_=w2_d[r, s, c, :, :].rearrange("(u p) x -> p u x", u=2))
                    rmv = rsz.tile([112, 2, 2, W], fp16, tag="rmv")
                    nc.sync.dma_start(out=rmv[:], in_=rmov_d[sg, :, :, :, :])
                    for c in range(C):
                        t1t = rsz.tile([112, 2, 224], fp16, tag="t1t")
                        for mh in range(2):
                            acc = psp.tile([112, W], fp32, tag="acc")
                            for kh in range(2):
                                nc.tensor.matmul(
                                    acc[:],
                                    yrs[:, kh, c, mh * 112:(mh + 1) * 112],
                                    rmv[:, 0, kh, :],
                                    start=(kh == 0), stop=(kh == 1))
                            nc.scalar.copy(out=t1t[:, mh, :], in_=acc[:])
                        ost = rsz.tile([112, W], fp32, tag="ost")
                        for mh2 in range(2):
                            acc2 = psp.tile([112, W], fp32, tag="acc2")
                            for kh2 in range(2):
                                nc.tensor.matmul(
                                    acc2[:],
                                    t1t[:, kh2, mh2 * 112:(mh2 + 1) * 112],
                                    rmv[:, 1, kh2, :],
                                    start=(kh2 == 0), stop=(kh2 == 1))
                            nc.scalar.copy(out=ost[:], in_=acc2[:])
                            nc.sync.dma_start(
                                out=out_d[sg, c, mh2 * 112:(mh2 + 1) * 112, :], in_=ost[:])
    nc.compile()
    return nc


# ------------------------------------------------------------------ driver
def _host_pack(inputs):
    x = np.asarray(inputs['x'], np.float32)
    noise = np.asarray(inputs['noise'], np.float32)
    bright = np.asarray(inputs['brightness'], np.float32)
    flip = np.asarray(inputs['flip_mask'], np.int32)
    ep = np.asarray(inputs['ep_raw'], np.int32)
    ang = np.asarray(inputs['angles'], np.int32)
    cij = np.asarray(inputs['crop_ij'], np.int32)

    # prep: flip + noise + brightness (exact fp32 order as reference), cast fp16
    xf = np.where(flip[:, None, None, None] > 0, x[..., ::-1], x)
    y = (xf + np.float32(0.625) * noise) * (np.float32(0.85) + np.float32(0.30) * bright)[:, None, None, None]
    y16 = y.astype(np.float16)

    R190 = bicubic_weight_mat(CROP, H)     # [190, 224]

    per_core = []
    for core in range(NCORES):
        sl = slice(core * SPC, (core + 1) * SPC)
        ys = y16[sl]                       # [32, 3, 224, 224]
        # even/odd pair arrays
        yy = np.zeros((ROUNDS, 8, C, 2, H, W), np.float16)
        yy[:, :, :, 0] = ys.reshape(ROUNDS, 8, C, H, W)
        yy[:, :, :, 1, :, :W - 1] = ys.reshape(ROUNDS, 8, C, H, W)[..., 1:]

        idxC = np.zeros((2, ROUNDS, NBAND * NCH, 128, NI // 16), np.int16)
        wts = np.zeros((2, ROUNDS, NBAND * NCH, 2, 8, NI, 2), np.float16)
        for rr in range(ROUNDS):
            for s in range(8):
                sg = core * SPC + rr * 8 + s
                for w in range(2):
                    if w == 0:
                        sx, sy = persp_grid(ep[sg])
                    else:
                        sx, sy = rot_grid(ang[sg])
                    iC, wA, wB = pack_warp(sx, sy)
                    for ci in range(NBAND * NCH):
                        idxC[w, rr, ci, 16 * s:16 * s + 16, :] = wrap16(iC[ci])
                        wts[w, rr, ci, 0, s] = wA[ci]
                        wts[w, rr, ci, 1, s] = wB[ci]

        rmov = np.zeros((SPC, 112, 2, 2, W), np.float16)
        for si in range(SPC):
            sg = core * SPC + si
            i0, j0 = int(cij[sg, 0]), int(cij[sg, 1])
            Rh = np.zeros((H, H), np.float32)
            Rw = np.zeros((H, H), np.float32)
            Rh[i0:i0 + CROP, :] = R190
            Rw[j0:j0 + CROP, :] = R190
            # [p, hw, kh, W]: stationary k-chunk kh covers rows kh*112+p
            rmov[si, :, 0, :, :] = Rh.reshape(2, 112, W).transpose(1, 0, 2).astype(np.float16)
            rmov[si, :, 1, :, :] = Rw.reshape(2, 112, W).transpose(1, 0, 2).astype(np.float16)

        per_core.append({
            "y16": yy, "idxC": idxC, "wts": wts, "rmov": rmov,
        })
    return per_core


def _axon_shim():
    """Make trace=True work under axon (missing antenv.axon_hooks in image)
    and stub the artifact upload (zero-egress container)."""
    import types
    try:
        import antenv.axon_hooks  # noqa
    except ImportError:
        mod = types.ModuleType('antenv.axon_hooks')
        mod._hook = None
        mod.set_axon_ntff_profile_hook = lambda h: setattr(mod, '_hook', h)
        mod.get_axon_ntff_profile_hook = lambda: mod._hook
        sys.modules['antenv.axon_hooks'] = mod
        import antenv
        antenv.axon_hooks = mod
    from antenv.axon_hooks import get_axon_ntff_profile_hook, set_axon_ntff_profile_hook
    if get_axon_ntff_profile_hook() is None:
        try:
            from trn_agent_boot.trn_boot import _ntff_profile_via_ctypes
            set_axon_ntff_profile_hook(_ntff_profile_via_ctypes('/opt/axon/libaxon_pjrt.so'))
        except Exception:
            pass
    from concourse import bass_utils
    bass_utils.upload_artifacts = lambda tmpdir: f"local://{tmpdir}"


def kernel(**inputs):
    _axon_shim()
    from concourse import bass_utils

    per_core = _host_pack(inputs)
    if _NC_CACHE[0] is None:
        _NC_CACHE[0] = build_nc()
    nc = _NC_CACHE[0]

    import os
    trace = bool(int(os.environ.get("KERNEL_TRACE", "0")))
    res = bass_utils.run_bass_kernel_spmd(
        nc, per_core, list(range(NCORES)), trace=trace)
    if trace and res.exec_time_ns is not None:
        print(f"HW exec time: {res.exec_time_ns} ns")
        kernel.last_exec_ns = res.exec_time_ns
    out = np.concatenate([res.results[i]["outp"] for i in range(NCORES)], axis=0)
    return out.astype(np.float32)

